# revision 23
# baseline (speedup 1.0000x reference)
# LoftQ fused kernel for Trainium2 (Bass/Tile), 8-core data-parallel, fp8.
#
# reference:
#   W_q = (W_int - zero_point) * scale                  [out=4096, in=4096]
#   W   = W_q + (lora_B @ lora_A) * RANK**-0.5
#   y   = einsum('bsd,od->bso', x, W)                   x: [4, 2048, 4096]
#
# Strategy:
#   - Data-parallel: 8192 tokens sharded 1024/core; W replicated.
#   - Decompose y = s*(x @ W_int.T) - s*zp*rowsum(x) + (x @ A.T) @ (sc*B.T)
#     W_int values 0..15 are EXACT in fp8e4m3, so the main GEMM runs as
#     fp8 x fp8 with MatmulPerfMode.DoubleRow (K=256 per instruction,
#     ~216ns issue rate per [256x128x512] matmul; ~245us of PE work/core).
#   - x is split hi/lo: xhi = f8(x), xlo = f8(16*(x - xhi)). The main GEMM
#     uses xhi only (its error lands on the small quantized term; the
#     LoRA term dominates output variance). The LoRA path u = x @ A_aug.T
#     runs as fp8 DoubleRow chains: chain12 = xhi @ [Ahi ; 16*(A-Ahi)]
#     plus chain3 = xlo @ (Ahi/16), giving ~bf16 accuracy.
#   - zero point folded in by augmenting A with a ones row; bts row 16 =
#     -zp. A K=64 bf16 tail matmul per oc-tile adds u @ (sc*B.T/s | -zp)
#     into the main PSUM group; eviction scales by s.
#
# DMA system model (from trace archaeology): two HWDGE rings (sync=SP,
# scalar=ACT) issue in program order, ~0.6us engine time per dma_start,
# sharing 8 global completion-semaphore lanes with ONE outstanding DMA
# per lane (round-robin in combined issue order) and ~1.5us fixed
# latency per piece; aggregate effective bandwidth during bursts is
# ~270-330 GB/s. Choreography is therefore:
#   - Ring split by role: sync ring = x-side + y writebacks, scalar ring
#     = W only, so W pieces are never head-of-line blocked behind x
#     queue-drain waits (that caused an 8.3us PE stall at a W-group
#     boundary when everything shared one ring).
#   - oc-chunk-major main loop (one 512-wide PSUM bank per tile, 6 in
#     flight): the startup only needs ONE 2MB W chunk, and each W chunk
#     thereafter has a ~30us landing window (67 GB/s steady demand).
#   - Startup interleave: per dt2 block the PE runs chain12 (both token
#     halves), chain3 for token-half 0 (lagged 4 blocks so xlo pieces
#     have landing slack), and the first 6 token-tiles' DR steps for oc
#     chunk 0 — consuming x/W pieces as they land (~270 GB/s demand vs
#     the ~1.9us/block PE pace) instead of idling until xhi is resident.
#   - xlo is packed token-half-major; chain3 for half 1 closes later,
#     behind the half-0 tails.
#   - Pieces are deadline-ordered and sized ~256-512KB-1MB (small enough
#     to pace consumption, big enough to amortize the per-piece latency;
#     the first xhi block is a single piece so no lane straggler gates
#     the first matmul).
#   - W chunks 1+ are gated behind xlo landings (a dummy scalar read of
#     the last xlo piece) so startup SDMA bandwidth goes to x; chunks 3+
#     auto-gate on W-pool buffer reuse (3 resident 2MB buffers).
#   - All PSUM evictions run on the vector engine (scalar is the W DMA
#     issuer). The last tile's writeback is serialized and split across
#     both HWDGE rings to shorten the drain.
#
# Host-side work is limited to sharding/layout packing (transpose + dtype
# packing); all FLOPs (both matmuls, dequant-by-linearity) run on device.

import numpy as np
import ml_dtypes

import concourse.bass as bass
import concourse.mybir as mybir
import concourse.tile as tile
from concourse import bacc
from concourse.bass import ts
from concourse.bass_utils import run_bass_kernel_spmd

P = 128
N_CORES = 8
RANK = 16
# u-phase PSUM row layout: [u_top(17); u_top duplicate(17); u_bot(17); pad]
# - u_top = x @ A_aug.T (A augmented with a ones row for the zp rowsum)
# - the duplicate rows are free (PE cost is moving-row-bound) and make
#   every fp8 eviction slice partition-aligned for the dual-fp8 tail
# - u_bot = xhi @ (16*(A - A_hi)).T, the A-quantization correction
RT = RANK + 1
R2 = 2 * RT
R3 = 3 * RT
RB = 64   # u-phase PSUM rows
RC = 48   # chain3 stationary cols: R2 padded — dual-fp8 ldweights needs
          # the k-pair stride (the lhsT column count) % 16 == 0
KB = 128  # tail stationary partitions: full 128 measures ~100ns faster
          # per tail MM than a 64-partition stationary (same ap cost)
SCALING = RANK ** (-0.5)
BF16 = mybir.dt.bfloat16
F32 = mybir.dt.float32
F8 = mybir.dt.float8e4
OC = 512      # output-feature chunk (one PSUM bank wide)

DR = mybir.MatmulPerfMode.DoubleRow
COPY = mybir.ActivationFunctionType.Copy
MULT = mybir.AluOpType.mult


def build_program(nc, T, D, O, scale):
    """Emit the per-core program.

    T: tokens per core, D: in_features, O: out_features.
    Inputs (per core):
      xhi  f8e4 [P, D/P, T]        f8(x) shard, transposed+partition-packed
      xlo  f8e4 [P, NHB, D/P, UW]  f8(16*(x - xhi)), token-half-major
      w8c  f8e4 [NOC, P, D/P, OC]  W_int^T chunk-packed (replicated)
      a2   f8e4 [P, D/P, RB]  [Ahi(17) ; Ahi(17) ; 16*(A_aug-Ahi)(17) ; 0]
      a16  f8e4 [P, D/P, RC]  [Ahi/16(17) ; Ahi/16(17) ; 0-pad] (replicated)
      bts8 f8e4 [KB, 2, O]    dual-fp8 B-side for the DR tail (replicated)
    Output: y bf16 [T, O]  (host casts to f32); y = scale * psum
    """
    DT, TT, NOC = D // P, T // P, O // OC
    D2 = DT // 2
    UW = min(512, T)     # u-phase moving width (one token half)
    NHB = T // UW
    TPH = UW // P        # token tiles per half
    assert DT % 2 == 0

    xhi = nc.dram_tensor("xhi", [P, DT, T], F8, kind="ExternalInput")
    xlo = nc.dram_tensor("xlo", [P, NHB, DT, UW], F8, kind="ExternalInput")
    w8 = nc.dram_tensor("w8c", [NOC, P, DT, OC], F8, kind="ExternalInput")
    a2 = nc.dram_tensor("a2", [P, DT, RB], F8, kind="ExternalInput")
    a16 = nc.dram_tensor("a16", [P, DT, RC], F8, kind="ExternalInput")
    bts8 = nc.dram_tensor("bts8", [RB, 2, O], F8, kind="ExternalInput")
    y = nc.dram_tensor("y", [T, O], BF16, kind="ExternalOutput")
    y_ap = y.ap().rearrange("(tt p) o -> p tt o", p=P)

    iters = [(oc, tt) for oc in range(NOC) for tt in range(TT)]
    PRE = max(1, min(6, len(iters) - 1))
    CH3_LAG = min(6, max(1, D2 - 1))

    with tile.TileContext(nc) as tc:
        with (
            tc.tile_pool(name="const", bufs=1) as cpool,
            tc.tile_pool(name="wc", bufs=min(3, NOC)) as wcpool,
            tc.tile_pool(name="outpool", bufs=4) as outpool,
            tc.tile_pool(name="psum", bufs=6, space="PSUM") as psum,
            tc.tile_pool(name="psum_u", bufs=NHB, space="PSUM") as psum_u,
        ):
            a2_sb = cpool.tile([P, DT, RB], F8)
            xhi_sb = cpool.tile([P, DT, T], F8)
            xlo_sb = cpool.tile([P, NHB, DT, UW], F8)
            a16_sb = cpool.tile([P, DT, RC], F8)
            bts8_sb = cpool.tile([KB, 2, O], F8)
            u8_sb = cpool.tile([KB, 2, T], F8)
            uhf_sb = cpool.tile([RT, UW], F32)
            uc_sb = cpool.tile([R3, UW], F32)
            gate_sb = cpool.tile([1, 16], BF16)
            # zero the u8 pad slots once (a stale NaN times a zero bts8
            # row would poison the tail accumulation); bts8's upper rows
            # are zero too and never DMA'd (saves 512KB in the startup
            # bandwidth crunch)
            nc.gpsimd.memset(u8_sb[:], 0.0)
            nc.gpsimd.memset(bts8_sb[RB:KB], 0.0)
            wc_sb = [
                wcpool.tile([P, DT, OC], F8, tag="wc", name=f"wc_{oc}")
                for oc in range(NOC)
            ]

            # ---- sync (SP) ring: x-side inputs, deadline-ordered ----
            # xhi's first block is the true gate for the first matmul;
            # the tiny a2 piece lands fast from any slot
            nc.sync.dma_start(xhi_sb[:, 0:2], xhi.ap()[:, 0:2])
            nc.sync.dma_start(a2_sb[:, 0:2], a2.ap()[:, 0:2])
            nc.sync.dma_start(a16_sb[:], a16.ap())
            if DT > 2:
                nc.sync.dma_start(a2_sb[:, 2:DT], a2.ap()[:, 2:DT])
                nc.sync.dma_start(xhi_sb[:, 2:4], xhi.ap()[:, 2:4])
            # remaining xhi in 4-dt pieces, interleaved with xlo-half-0
            # pieces so both streams meet the phase-S consumption pace
            xhp = [(d, min(d + 4, DT)) for d in range(4, DT, 4)]
            NXP = 4 if DT % 8 == 0 else 1
            XD = DT // NXP

            def xlo_pieces(hb):
                return [(hb, r * XD, (r + 1) * XD) for r in range(NXP)]

            lq = list(xlo_pieces(0))
            # deadline order: chain3 lags 8 blocks, so xlo-h0 pieces slot
            # in after every third xhi piece (xhi+W gate 6 of 8 MMs/step)
            for i, (d0, d1) in enumerate(xhp):
                nc.sync.dma_start(xhi_sb[:, d0:d1], xhi.ap()[:, d0:d1])
                if i % 2 == 1 and lq:
                    hb, l0, l1 = lq.pop(0)
                    nc.sync.dma_start(
                        xlo_sb[:, hb, l0:l1], xlo.ap()[:, hb, l0:l1]
                    )
            for hb, l0, l1 in lq:
                nc.sync.dma_start(xlo_sb[:, hb, l0:l1], xlo.ap()[:, hb, l0:l1])
            nc.sync.dma_start(bts8_sb[0:RB], bts8.ap())
            for hb in range(1, NHB):
                for _, l0, l1 in xlo_pieces(hb):
                    nc.sync.dma_start(
                        xlo_sb[:, hb, l0:l1], xlo.ap()[:, hb, l0:l1]
                    )

            # ---- scalar (ACT) ring: W only ----
            pre_ocs = []
            for oc, _ in iters[:PRE]:
                if oc not in pre_ocs:
                    pre_ocs.append(oc)
            rest_ocs = []
            for oc, _ in iters[PRE:]:
                if oc not in pre_ocs and oc not in rest_ocs:
                    rest_ocs.append(oc)
            NWP = 4 if DT % 8 == 0 else 1
            WD = DT // NWP
            for oc in pre_ocs:
                for r in range(NWP):
                    nc.scalar.dma_start(
                        wc_sb[oc][:, r * WD : (r + 1) * WD],
                        w8.ap()[oc, :, r * WD : (r + 1) * WD],
                    )
            for k, oc in enumerate(rest_ocs):
                if k == 0:
                    # gate: consume the last piece of xlo-half-0
                    nc.scalar.activation(
                        gate_sb[:], xlo_sb[0:1, 0, DT - 1, 0:16], COPY
                    )
                if k == 1 and NHB > 1:
                    nc.scalar.activation(
                        gate_sb[:], xlo_sb[0:1, NHB - 1, DT - 1, 0:16], COPY
                    )
                for h in range(2):
                    d0, d1 = h * DT // 2, (h + 1) * DT // 2
                    nc.scalar.dma_start(
                        wc_sb[oc][:, d0:d1], w8.ap()[oc, :, d0:d1]
                    )

            # ---- PE program ----
            pu = [
                psum_u.tile([RB, UW], F32, tag="pu", name=f"pu_{hb}")
                for hb in range(NHB)
            ]
            ps = {}

            def u12_mm(dt2, hb, stop=False):
                nc.tensor.matmul(
                    pu[hb][:RB],
                    lhsT=a2_sb[:, 2 * dt2 : 2 * dt2 + 2],
                    rhs=xhi_sb[:, 2 * dt2 : 2 * dt2 + 2, ts(hb, UW)],
                    start=(dt2 == 0),
                    stop=stop,
                    perf_mode=DR,
                )

            def u3_mm(hb, dt2):
                nc.tensor.matmul(
                    pu[hb][:RC],
                    lhsT=a16_sb[:, 2 * dt2 : 2 * dt2 + 2],
                    rhs=xlo_sb[:, hb, 2 * dt2 : 2 * dt2 + 2],
                    start=False,
                    stop=False,
                    perf_mode=DR,
                )

            def prep_u8(hb):
                # dual-fp8 u for the DR tail: one partition-aligned copy
                # covers u_top-hi, its duplicate, and u_bot; the residual
                # ul = u_top - f8(u_top) fills the pair slot of rows 0:RT.
                # TRN fp8e4 overflows to inf above +-240 (not the OCP 448),
                # so clamp before the downcast; the ul residual is computed
                # against the clamped+rounded value, so it absorbs the
                # clamp error exactly on the uh*Bh path.
                hs = ts(hb, UW)
                nc.vector.tensor_scalar_min(uc_sb[:], pu[hb][0:R3], 224.0)
                nc.vector.tensor_scalar_max(uc_sb[:], uc_sb[:], -224.0)
                nc.vector.tensor_scalar(
                    u8_sb[0:R3, 0, hs], uc_sb[:], 1.0, None, MULT
                )
                nc.vector.tensor_scalar(
                    uhf_sb[:], u8_sb[0:RT, 0, hs], 1.0, None, MULT
                )
                nc.vector.tensor_tensor(
                    u8_sb[0:RT, 1, hs], pu[hb][0:RT], uhf_sb[:],
                    mybir.AluOpType.subtract,
                )

            def emit_dr_step(oc, tt, dt2):
                if dt2 == 0:
                    ps[oc, tt] = psum.tile(
                        [P, OC], F32, tag="ps", name=f"ps_{oc}_{tt}"
                    )
                nc.tensor.matmul(
                    ps[oc, tt][:],
                    lhsT=xhi_sb[:, 2 * dt2 : 2 * dt2 + 2, ts(tt, P)],
                    rhs=wc_sb[oc][:, 2 * dt2 : 2 * dt2 + 2],
                    start=(dt2 == 0),
                    stop=False,
                    perf_mode=DR,
                )

            def emit_drs(oc, tt):
                for dt2 in range(D2):
                    emit_dr_step(oc, tt, dt2)

            def emit_tail_evict(oc, tt, nsp, alt_ring=False):
                nc.tensor.matmul(
                    ps[oc, tt][:],
                    lhsT=u8_sb[:, :, ts(tt, P)],
                    rhs=bts8_sb[:, :, ts(oc, OC)],
                    start=False,
                    stop=True,
                    perf_mode=DR,
                )
                ob = outpool.tile([P, OC], BF16, tag="ob", name=f"ob_{oc}_{tt}")
                nc.vector.tensor_scalar(ob[:], ps[oc, tt][:], scale, None, MULT)
                for q in range(nsp):
                    pr = ts(q, P // nsp)
                    eng = nc.scalar if (alt_ring and (oc + tt) % 2) else nc.sync
                    eng.dma_start(y_ap[pr, tt, ts(oc, OC)], ob[pr])

            # Startup interleave: per dt2 block the u chains and the first
            # PRE tiles' DR steps consume x/W pieces as each block lands.
            for dt2 in range(D2):
                if dt2 < D2 - 1:
                    for hb in range(NHB):
                        u12_mm(dt2, hb)
                if dt2 >= CH3_LAG:
                    u3_mm(0, dt2 - CH3_LAG)
                for oc, tt in iters[:PRE]:
                    emit_dr_step(oc, tt, dt2)
            for dt2 in range(D2 - CH3_LAG, D2):
                u3_mm(0, dt2)
            u12_mm(D2 - 1, 0, stop=True)
            prep_u8(0)
            # token-half-0 tails first; close half 1 behind them
            for oc, tt in iters[:PRE]:
                if tt < TPH:
                    emit_tail_evict(oc, tt, 1)
            for hb in range(1, NHB):
                for dt2 in range(D2):
                    u3_mm(hb, dt2)
                u12_mm(D2 - 1, hb, stop=True)
                prep_u8(hb)
            for oc, tt in iters[:PRE]:
                if tt >= TPH:
                    emit_tail_evict(oc, tt, 1)
            # steady state: batch 6 tiles' DR chains then their 6 tails —
            # fewer chunk boundaries (~400ns ldweights refill each); the
            # first DR of chunk i waits only on chunk i-1's first
            # eviction, which completes while the remaining tails issue
            GRP = 6
            rest = iters[PRE:-1]
            for c0 in range(0, len(rest), GRP):
                chunk = rest[c0 : c0 + GRP]
                for oc, tt in chunk:
                    emit_drs(oc, tt)
                for oc, tt in chunk:
                    emit_tail_evict(oc, tt, 1)
            # final tile: the last writeback is evicted in partition
            # halves and split across both HWDGE rings to cut the drain.
            if PRE < len(iters):
                ocL, ttL = iters[-1]
                emit_drs(ocL, ttL)
                nc.tensor.matmul(
                    ps[ocL, ttL][:],
                    lhsT=u8_sb[:, :, ts(ttL, P)],
                    rhs=bts8_sb[:, :, ts(ocL, OC)],
                    start=False,
                    stop=True,
                    perf_mode=DR,
                )
                ob = outpool.tile([P, OC], BF16, tag="ob", name="ob_last")
                for h in range(2):
                    hr = ts(h, P // 2)
                    nc.vector.tensor_scalar(
                        ob[hr], ps[ocL, ttL][hr], scale, None, MULT
                    )
                    q0 = ts(2 * h, P // 4)
                    q1 = ts(2 * h + 1, P // 4)
                    nc.sync.dma_start(y_ap[q0, ttL, ts(ocL, OC)], ob[q0])
                    nc.scalar.dma_start(y_ap[q1, ttL, ts(ocL, OC)], ob[q1])
    return nc


def _pack_inputs(x, W_int, lora_A, lora_B, scale, zero_point):
    """Host-side shard + layout packing. Returns per-core input maps."""
    F8NP = ml_dtypes.float8_e4m3
    BFNP = ml_dtypes.bfloat16
    BS, S, D = x.shape
    O = W_int.shape[0]
    Tfull = BS * S
    T = Tfull // N_CORES
    DT = D // P
    NOC = O // OC
    UW = min(512, T)
    NHB = T // UW
    s = float(scale)
    zp = float(zero_point)

    def pack_x(v):  # [T, D] -> [P, DT, T]
        return np.ascontiguousarray(v.T.reshape(DT, P, T).transpose(1, 0, 2))

    def pack_x_hb(v):  # [T, D] -> [P, NHB, DT, UW] token-half-major
        return np.ascontiguousarray(
            v.reshape(NHB, UW, DT, P).transpose(3, 0, 2, 1)
        )

    xf = np.asarray(x, dtype=np.float32).reshape(Tfull, D)
    # [oc, p, dt, j] <- W_int^T[d=dt*P+p, o=oc*OC+j], exact in fp8e4m3
    w8c = np.ascontiguousarray(
        np.asarray(W_int, dtype=np.float32)
        .astype(F8NP)
        .T.reshape(DT, P, NOC, OC)
        .transpose(2, 1, 0, 3)
    )
    A_aug = np.concatenate(
        [
            np.asarray(lora_A, dtype=np.float32),
            np.ones((1, D), np.float32),
        ],
        axis=0,
    )  # [RT, D]

    def pack_a(v):  # [R, D] -> [P, DT, R]
        R = v.shape[0]
        return np.ascontiguousarray(v.T.reshape(DT, P, R).transpose(1, 0, 2).astype(F8NP))

    A_hi = A_aug.astype(F8NP).astype(np.float32)
    A_lo16 = ((A_aug - A_hi) * 16.0).astype(F8NP).astype(np.float32)
    # pu row layout: [u_top(RT); u_top dup(RT); u_bot(RT); pad to RB]
    a2 = pack_a(
        np.concatenate(
            [A_hi, A_hi, A_lo16, np.zeros((RB - R3, D), np.float32)], axis=0
        )
    )
    a16 = pack_a(
        np.concatenate(
            [A_hi / 16.0, A_hi / 16.0, np.zeros((RC - R2, D), np.float32)],
            axis=0,
        )
    )
    # dual-fp8 B-side [RB, 2, O]: (p, j) slots pair with u8 as
    #   rows 0:RT   j=0: uh*Bh     j=1: ul*Bh
    #   rows RT:R2  j=0: uh2*Bl    j=1: 0
    #   rows R2:R3  j=0: ub*(Bh/16) j=1: 0      (u_bot carries a 16x)
    Bp = np.concatenate(
        [
            np.asarray(lora_B, dtype=np.float32).T * (SCALING / s),
            np.full((1, O), -zp, np.float32),
        ],
        axis=0,
    )  # [RT, O]
    Bh8 = Bp.astype(F8NP)
    Bhf = Bh8.astype(np.float32)
    Bl8 = (Bp - Bhf).astype(F8NP)
    bts8 = np.zeros((RB, 2, O), F8NP)
    bts8[0:RT, 0] = Bh8
    bts8[0:RT, 1] = Bh8
    bts8[RT:R2, 0] = Bl8
    bts8[R2:R3, 0] = (Bhf / 16.0).astype(F8NP)
    bts8 = np.ascontiguousarray(bts8)
    in_maps = []
    for c in range(N_CORES):
        xs = xf[c * T : (c + 1) * T]  # [T, D] f32
        xhi8 = xs.astype(F8NP)
        xlo8 = ((xs - xhi8.astype(np.float32)) * 16.0).astype(F8NP)
        in_maps.append(
            {
                "xhi": pack_x(xhi8),
                "xlo": pack_x_hb(xlo8),
                "w8c": w8c,
                "a2": a2,
                "a16": a16,
                "bts8": bts8,
            }
        )
    return in_maps, T, D, O


def _install_ntff_shim():
    """Provide antenv.axon_hooks (absent in this image) so that
    run_bass_kernel_spmd(trace=True) can capture NTFF profiles via the
    axon .so — mirrors trn_agent_boot.trn_boot's degraded-silently path.
    Only used for our own measurement runs (_trace=True)."""
    import sys as _sys
    import types as _types

    if "antenv.axon_hooks" in _sys.modules:
        return
    try:
        from trn_agent_boot.trn_boot import _ntff_profile_via_ctypes
    except ImportError:
        _sys.path.insert(0, "/root/.axon_site")
        from trn_agent_boot.trn_boot import _ntff_profile_via_ctypes

    hook = _ntff_profile_via_ctypes("/opt/axon/libaxon_pjrt.so")
    mod = _types.ModuleType("antenv.axon_hooks")
    mod._hook = hook
    mod.get_axon_ntff_profile_hook = lambda: mod._hook
    mod.set_axon_ntff_profile_hook = lambda h: setattr(mod, "_hook", h)
    _sys.modules["antenv.axon_hooks"] = mod
    import antenv as _antenv

    _antenv.axon_hooks = mod


def kernel(x, W_int, lora_A, lora_B, scale, zero_point, _trace=False, _tmpdir=None):
    if _trace:
        _install_ntff_shim()
    x = np.asarray(x)
    BS, S, D = x.shape
    s = float(np.asarray(scale))
    zp = float(np.asarray(zero_point))
    in_maps, T, D, O = _pack_inputs(x, W_int, lora_A, lora_B, s, zp)

    nc = bacc.Bacc(
        "TRN2",
        target_bir_lowering=False,
        debug=False,
        num_devices=N_CORES,
    )
    build_program(nc, T, D, O, scale=s)
    nc.compile()

    res = run_bass_kernel_spmd(
        nc,
        in_maps,
        core_ids=list(range(N_CORES)),
        trace=_trace,
        tmpdir=_tmpdir,
        trace_cores=list(range(N_CORES)) if _trace else None,
    )
    y = (
        np.concatenate([np.asarray(r["y"]) for r in res.results], axis=0)
        .astype(np.float32)
        .reshape(BS, S, O)
    )
    if _trace:
        kernel.last_results = res
    return y


if __name__ == "__main__":
    # smoke: build-only for full shapes
    nc = bacc.Bacc("TRN2", target_bir_lowering=False, debug=False, num_devices=8)
    build_program(nc, 1024, 4096, 4096, scale=0.01)
    nc.compile()
    print("build ok; instructions:", sum(len(b.instructions) for b in nc.main_func.blocks))


# revision 24
# speedup vs baseline: 1.0123x; 1.0123x over previous
# LoftQ fused kernel for Trainium2 (Bass/Tile), 8-core data-parallel, fp8.
#
# reference:
#   W_q = (W_int - zero_point) * scale                  [out=4096, in=4096]
#   W   = W_q + (lora_B @ lora_A) * RANK**-0.5
#   y   = einsum('bsd,od->bso', x, W)                   x: [4, 2048, 4096]
#
# Strategy:
#   - Data-parallel: 8192 tokens sharded 1024/core; W replicated.
#   - Decompose y = s*(x @ W_int.T) - s*zp*rowsum(x) + (x @ A.T) @ (sc*B.T)
#     W_int values 0..15 are EXACT in fp8e4m3, so the main GEMM runs as
#     fp8 x fp8 with MatmulPerfMode.DoubleRow (K=256 per instruction,
#     ~216ns issue rate per [256x128x512] matmul; ~245us of PE work/core).
#   - x is split hi/lo: xhi = f8(x), xlo = f8(16*(x - xhi)). The main GEMM
#     uses xhi only (its error lands on the small quantized term; the
#     LoRA term dominates output variance). The LoRA path u = x @ A_aug.T
#     runs as fp8 DoubleRow chains: chain12 = xhi @ [Ahi ; 16*(A-Ahi)]
#     plus chain3 = xlo @ (Ahi/16), giving ~bf16 accuracy.
#   - zero point folded in by augmenting A with a ones row; bts row 16 =
#     -zp. A K=64 bf16 tail matmul per oc-tile adds u @ (sc*B.T/s | -zp)
#     into the main PSUM group; eviction scales by s.
#
# DMA system model (from trace archaeology): two HWDGE rings (sync=SP,
# scalar=ACT) issue in program order, ~0.6us engine time per dma_start,
# sharing 8 global completion-semaphore lanes with ONE outstanding DMA
# per lane (round-robin in combined issue order) and ~1.5us fixed
# latency per piece; aggregate effective bandwidth during bursts is
# ~270-330 GB/s. Choreography is therefore:
#   - Ring split by role: sync ring = x-side + y writebacks, scalar ring
#     = W only, so W pieces are never head-of-line blocked behind x
#     queue-drain waits (that caused an 8.3us PE stall at a W-group
#     boundary when everything shared one ring).
#   - oc-chunk-major main loop (one 512-wide PSUM bank per tile, 6 in
#     flight): the startup only needs ONE 2MB W chunk, and each W chunk
#     thereafter has a ~30us landing window (67 GB/s steady demand).
#   - Startup interleave: per dt2 block the PE runs chain12 (both token
#     halves), chain3 for token-half 0 (lagged 4 blocks so xlo pieces
#     have landing slack), and the first 6 token-tiles' DR steps for oc
#     chunk 0 — consuming x/W pieces as they land (~270 GB/s demand vs
#     the ~1.9us/block PE pace) instead of idling until xhi is resident.
#   - xlo is packed token-half-major; chain3 for half 1 closes later,
#     behind the half-0 tails.
#   - Pieces are deadline-ordered and sized ~256-512KB-1MB (small enough
#     to pace consumption, big enough to amortize the per-piece latency;
#     the first xhi block is a single piece so no lane straggler gates
#     the first matmul).
#   - W chunks 1+ are gated behind xlo landings (a dummy scalar read of
#     the last xlo piece) so startup SDMA bandwidth goes to x; chunks 3+
#     auto-gate on W-pool buffer reuse (3 resident 2MB buffers).
#   - All PSUM evictions run on the vector engine (scalar is the W DMA
#     issuer). The last tile's writeback is serialized and split across
#     both HWDGE rings to shorten the drain.
#
# Host-side work is limited to sharding/layout packing (transpose + dtype
# packing); all FLOPs (both matmuls, dequant-by-linearity) run on device.

import numpy as np
import ml_dtypes

import concourse.bass as bass
import concourse.mybir as mybir
import concourse.tile as tile
from concourse import bacc
from concourse.bass import ts
from concourse.bass_utils import run_bass_kernel_spmd

P = 128
N_CORES = 8
RANK = 16
# u-phase PSUM row layout: [u_top(17); u_top duplicate(17); u_bot(17); pad]
# - u_top = x @ A_aug.T (A augmented with a ones row for the zp rowsum)
# - the duplicate rows are free (PE cost is moving-row-bound) and make
#   every fp8 eviction slice partition-aligned for the dual-fp8 tail
# - u_bot = xhi @ (16*(A - A_hi)).T, the A-quantization correction
RT = RANK + 1
R2 = 2 * RT
R3 = 3 * RT
RB = 64   # u-phase PSUM rows
RC = 48   # chain3 stationary cols: R2 padded — dual-fp8 ldweights needs
          # the k-pair stride (the lhsT column count) % 16 == 0
KB = 128  # tail stationary partitions: full 128 measures ~100ns faster
          # per tail MM than a 64-partition stationary (same ap cost)
SCALING = RANK ** (-0.5)
BF16 = mybir.dt.bfloat16
F32 = mybir.dt.float32
F8 = mybir.dt.float8e4
OC = 512      # output-feature chunk (one PSUM bank wide)

DR = mybir.MatmulPerfMode.DoubleRow
COPY = mybir.ActivationFunctionType.Copy
MULT = mybir.AluOpType.mult


def build_program(nc, T, D, O, scale):
    """Emit the per-core program.

    T: tokens per core, D: in_features, O: out_features.
    Inputs (per core):
      xhi  f8e4 [P, D/P, T]        f8(x) shard, transposed+partition-packed
      xlo  f8e4 [P, NHB, D/P, UW]  f8(16*(x - xhi)), token-half-major
      w8c  f8e4 [NOC, P, D/P, OC]  W_int^T chunk-packed (replicated)
      a2   f8e4 [P, D/P, RB]  [Ahi(17) ; Ahi(17) ; 16*(A_aug-Ahi)(17) ; 0]
      a16  f8e4 [P, D/P, RC]  [Ahi/16(17) ; Ahi/16(17) ; 0-pad] (replicated)
      bts8 f8e4 [KB, 2, O]    dual-fp8 B-side for the DR tail (replicated)
    Output: y bf16 [T, O]  (host casts to f32); y = scale * psum
    """
    DT, TT, NOC = D // P, T // P, O // OC
    D2 = DT // 2
    UW = min(512, T)     # u-phase moving width (one token half)
    NHB = T // UW
    TPH = UW // P        # token tiles per half
    assert DT % 2 == 0

    xhi = nc.dram_tensor("xhi", [P, DT, T], F8, kind="ExternalInput")
    xlo = nc.dram_tensor("xlo", [P, NHB, DT, UW], F8, kind="ExternalInput")
    w8 = nc.dram_tensor("w8c", [NOC, P, DT, OC], F8, kind="ExternalInput")
    a2 = nc.dram_tensor("a2", [P, DT, RB], F8, kind="ExternalInput")
    a16 = nc.dram_tensor("a16", [P, DT, RC], F8, kind="ExternalInput")
    bts8 = nc.dram_tensor("bts8", [RB, 2, O], F8, kind="ExternalInput")
    y = nc.dram_tensor("y", [T, O], BF16, kind="ExternalOutput")
    y_ap = y.ap().rearrange("(tt p) o -> p tt o", p=P)

    iters = [(oc, tt) for oc in range(NOC) for tt in range(TT)]
    PRE = max(1, min(6, len(iters) - 1))
    CH3_LAG = min(6, max(1, D2 - 1))

    with tile.TileContext(nc) as tc:
        with (
            tc.tile_pool(name="const", bufs=1) as cpool,
            tc.tile_pool(name="wc", bufs=min(3, NOC)) as wcpool,
            tc.tile_pool(name="outpool", bufs=4) as outpool,
            tc.tile_pool(name="psum", bufs=6, space="PSUM") as psum,
            tc.tile_pool(name="psum_u", bufs=NHB, space="PSUM") as psum_u,
        ):
            a2_sb = cpool.tile([P, DT, RB], F8)
            xhi_sb = cpool.tile([P, DT, T], F8)
            xlo_sb = cpool.tile([P, NHB, DT, UW], F8)
            a16_sb = cpool.tile([P, DT, RC], F8)
            bts8_sb = cpool.tile([KB, 2, O], F8)
            u8_sb = cpool.tile([KB, 2, T], F8)
            uhf_sb = cpool.tile([RT, UW], F32)
            uc_sb = cpool.tile([R3, UW], F32)
            gate_sb = cpool.tile([1, 16], BF16)
            # zero the u8 pad slots once (a stale NaN times a zero bts8
            # row would poison the tail accumulation); bts8's upper rows
            # are zero too and never DMA'd (saves 512KB in the startup
            # bandwidth crunch)
            nc.gpsimd.memset(u8_sb[:], 0.0)
            nc.gpsimd.memset(bts8_sb[RB:KB], 0.0)
            wc_sb = [
                wcpool.tile([P, DT, OC], F8, tag="wc", name=f"wc_{oc}")
                for oc in range(NOC)
            ]

            # ---- sync (SP) ring: x-side inputs, deadline-ordered ----
            # xhi's first block is the true gate for the first matmul;
            # the tiny a2 piece lands fast from any slot
            nc.sync.dma_start(xhi_sb[:, 0:2], xhi.ap()[:, 0:2])
            nc.sync.dma_start(a2_sb[:, 0:2], a2.ap()[:, 0:2])
            nc.sync.dma_start(a16_sb[:], a16.ap())
            if DT > 2:
                nc.sync.dma_start(a2_sb[:, 2:DT], a2.ap()[:, 2:DT])
                nc.sync.dma_start(xhi_sb[:, 2:4], xhi.ap()[:, 2:4])
            # remaining xhi in 4-dt pieces, interleaved with xlo-half-0
            # pieces so both streams meet the phase-S consumption pace
            xhp = [(d, min(d + 4, DT)) for d in range(4, DT, 4)]
            NXP = 4 if DT % 8 == 0 else 1
            XD = DT // NXP

            def xlo_pieces(hb):
                return [(hb, r * XD, (r + 1) * XD) for r in range(NXP)]

            lq = list(xlo_pieces(0))
            # deadline order: chain3 lags 8 blocks, so xlo-h0 pieces slot
            # in after every third xhi piece (xhi+W gate 6 of 8 MMs/step)
            for i, (d0, d1) in enumerate(xhp):
                nc.sync.dma_start(xhi_sb[:, d0:d1], xhi.ap()[:, d0:d1])
                if i % 2 == 1 and lq:
                    hb, l0, l1 = lq.pop(0)
                    nc.sync.dma_start(
                        xlo_sb[:, hb, l0:l1], xlo.ap()[:, hb, l0:l1]
                    )
            for hb, l0, l1 in lq:
                nc.sync.dma_start(xlo_sb[:, hb, l0:l1], xlo.ap()[:, hb, l0:l1])
            nc.sync.dma_start(bts8_sb[0:RB], bts8.ap())
            for hb in range(1, NHB):
                for _, l0, l1 in xlo_pieces(hb):
                    nc.sync.dma_start(
                        xlo_sb[:, hb, l0:l1], xlo.ap()[:, hb, l0:l1]
                    )

            # ---- scalar (ACT) ring: W only ----
            pre_ocs = []
            for oc, _ in iters[:PRE]:
                if oc not in pre_ocs:
                    pre_ocs.append(oc)
            rest_ocs = []
            for oc, _ in iters[PRE:]:
                if oc not in pre_ocs and oc not in rest_ocs:
                    rest_ocs.append(oc)
            NWP = 4 if DT % 8 == 0 else 1
            WD = DT // NWP
            # hold W off the initial HBM burst: the first tile DR needs
            # W only at ~+14us, while xhi's first block gates everything
            nc.scalar.activation(gate_sb[:], a2_sb[0:1, 1, 0:16], COPY)
            for oc in pre_ocs:
                for r in range(NWP):
                    nc.scalar.dma_start(
                        wc_sb[oc][:, r * WD : (r + 1) * WD],
                        w8.ap()[oc, :, r * WD : (r + 1) * WD],
                    )
            for k, oc in enumerate(rest_ocs):
                if k == 0:
                    # gate: consume the last piece of xlo-half-0
                    nc.scalar.activation(
                        gate_sb[:], xlo_sb[0:1, 0, DT - 1, 0:16], COPY
                    )
                if k == 1 and NHB > 1:
                    nc.scalar.activation(
                        gate_sb[:], xlo_sb[0:1, NHB - 1, DT - 1, 0:16], COPY
                    )
                for h in range(2):
                    d0, d1 = h * DT // 2, (h + 1) * DT // 2
                    nc.scalar.dma_start(
                        wc_sb[oc][:, d0:d1], w8.ap()[oc, :, d0:d1]
                    )

            # ---- PE program ----
            pu = [
                psum_u.tile([RB, UW], F32, tag="pu", name=f"pu_{hb}")
                for hb in range(NHB)
            ]
            ps = {}

            def u12_mm(dt2, hb, stop=False):
                nc.tensor.matmul(
                    pu[hb][:RB],
                    lhsT=a2_sb[:, 2 * dt2 : 2 * dt2 + 2],
                    rhs=xhi_sb[:, 2 * dt2 : 2 * dt2 + 2, ts(hb, UW)],
                    start=(dt2 == 0),
                    stop=stop,
                    perf_mode=DR,
                )

            def u3_mm(hb, dt2):
                nc.tensor.matmul(
                    pu[hb][:RC],
                    lhsT=a16_sb[:, 2 * dt2 : 2 * dt2 + 2],
                    rhs=xlo_sb[:, hb, 2 * dt2 : 2 * dt2 + 2],
                    start=False,
                    stop=False,
                    perf_mode=DR,
                )

            def prep_u8(hb):
                # dual-fp8 u for the DR tail: one partition-aligned copy
                # covers u_top-hi, its duplicate, and u_bot; the residual
                # ul = u_top - f8(u_top) fills the pair slot of rows 0:RT.
                # TRN fp8e4 overflows to inf above +-240 (not the OCP 448),
                # so clamp before the downcast; the ul residual is computed
                # against the clamped+rounded value, so it absorbs the
                # clamp error exactly on the uh*Bh path.
                hs = ts(hb, UW)
                nc.vector.tensor_scalar_min(uc_sb[:], pu[hb][0:R3], 224.0)
                nc.vector.tensor_scalar_max(uc_sb[:], uc_sb[:], -224.0)
                nc.vector.tensor_scalar(
                    u8_sb[0:R3, 0, hs], uc_sb[:], 1.0, None, MULT
                )
                nc.vector.tensor_scalar(
                    uhf_sb[:], u8_sb[0:RT, 0, hs], 1.0, None, MULT
                )
                nc.vector.tensor_tensor(
                    u8_sb[0:RT, 1, hs], pu[hb][0:RT], uhf_sb[:],
                    mybir.AluOpType.subtract,
                )

            def emit_dr_step(oc, tt, dt2):
                if dt2 == 0:
                    ps[oc, tt] = psum.tile(
                        [P, OC], F32, tag="ps", name=f"ps_{oc}_{tt}"
                    )
                nc.tensor.matmul(
                    ps[oc, tt][:],
                    lhsT=xhi_sb[:, 2 * dt2 : 2 * dt2 + 2, ts(tt, P)],
                    rhs=wc_sb[oc][:, 2 * dt2 : 2 * dt2 + 2],
                    start=(dt2 == 0),
                    stop=False,
                    perf_mode=DR,
                )

            def emit_drs(oc, tt):
                for dt2 in range(D2):
                    emit_dr_step(oc, tt, dt2)

            def emit_tail_evict(oc, tt, nsp, alt_ring=False):
                nc.tensor.matmul(
                    ps[oc, tt][:],
                    lhsT=u8_sb[:, :, ts(tt, P)],
                    rhs=bts8_sb[:, :, ts(oc, OC)],
                    start=False,
                    stop=True,
                    perf_mode=DR,
                )
                ob = outpool.tile([P, OC], BF16, tag="ob", name=f"ob_{oc}_{tt}")
                nc.vector.tensor_scalar(ob[:], ps[oc, tt][:], scale, None, MULT)
                for q in range(nsp):
                    pr = ts(q, P // nsp)
                    eng = nc.scalar if (alt_ring and (oc + tt) % 2) else nc.sync
                    eng.dma_start(y_ap[pr, tt, ts(oc, OC)], ob[pr])

            # Startup interleave: per dt2 block the u chains and the first
            # PRE tiles' DR steps consume x/W pieces as each block lands.
            for dt2 in range(D2):
                if dt2 < D2 - 1:
                    for hb in range(NHB):
                        u12_mm(dt2, hb)
                if dt2 >= CH3_LAG:
                    u3_mm(0, dt2 - CH3_LAG)
                for oc, tt in iters[:PRE]:
                    emit_dr_step(oc, tt, dt2)
            for dt2 in range(D2 - CH3_LAG, D2):
                u3_mm(0, dt2)
            u12_mm(D2 - 1, 0, stop=True)
            prep_u8(0)
            # token-half-0 tails first; close half 1 behind them
            for oc, tt in iters[:PRE]:
                if tt < TPH:
                    emit_tail_evict(oc, tt, 1)
            for hb in range(1, NHB):
                for dt2 in range(D2):
                    u3_mm(hb, dt2)
                u12_mm(D2 - 1, hb, stop=True)
                prep_u8(hb)
            for oc, tt in iters[:PRE]:
                if tt >= TPH:
                    emit_tail_evict(oc, tt, 1)
            # steady state: batch 6 tiles' DR chains then their 6 tails —
            # fewer chunk boundaries (~400ns ldweights refill each); the
            # first DR of chunk i waits only on chunk i-1's first
            # eviction, which completes while the remaining tails issue
            GRP = 6
            rest = iters[PRE:-1]
            for c0 in range(0, len(rest), GRP):
                chunk = rest[c0 : c0 + GRP]
                for oc, tt in chunk:
                    emit_drs(oc, tt)
                for oc, tt in chunk:
                    emit_tail_evict(oc, tt, 1)
            # final tile: the last writeback is evicted in partition
            # halves and split across both HWDGE rings to cut the drain.
            if PRE < len(iters):
                ocL, ttL = iters[-1]
                emit_drs(ocL, ttL)
                nc.tensor.matmul(
                    ps[ocL, ttL][:],
                    lhsT=u8_sb[:, :, ts(ttL, P)],
                    rhs=bts8_sb[:, :, ts(ocL, OC)],
                    start=False,
                    stop=True,
                    perf_mode=DR,
                )
                ob = outpool.tile([P, OC], BF16, tag="ob", name="ob_last")
                for h in range(2):
                    hr = ts(h, P // 2)
                    nc.vector.tensor_scalar(
                        ob[hr], ps[ocL, ttL][hr], scale, None, MULT
                    )
                    q0 = ts(2 * h, P // 4)
                    q1 = ts(2 * h + 1, P // 4)
                    nc.sync.dma_start(y_ap[q0, ttL, ts(ocL, OC)], ob[q0])
                    nc.scalar.dma_start(y_ap[q1, ttL, ts(ocL, OC)], ob[q1])
    return nc


def _pack_inputs(x, W_int, lora_A, lora_B, scale, zero_point):
    """Host-side shard + layout packing. Returns per-core input maps."""
    F8NP = ml_dtypes.float8_e4m3
    BFNP = ml_dtypes.bfloat16
    BS, S, D = x.shape
    O = W_int.shape[0]
    Tfull = BS * S
    T = Tfull // N_CORES
    DT = D // P
    NOC = O // OC
    UW = min(512, T)
    NHB = T // UW
    s = float(scale)
    zp = float(zero_point)

    def pack_x(v):  # [T, D] -> [P, DT, T]
        return np.ascontiguousarray(v.T.reshape(DT, P, T).transpose(1, 0, 2))

    def pack_x_hb(v):  # [T, D] -> [P, NHB, DT, UW] token-half-major
        return np.ascontiguousarray(
            v.reshape(NHB, UW, DT, P).transpose(3, 0, 2, 1)
        )

    xf = np.asarray(x, dtype=np.float32).reshape(Tfull, D)
    # [oc, p, dt, j] <- W_int^T[d=dt*P+p, o=oc*OC+j], exact in fp8e4m3
    w8c = np.ascontiguousarray(
        np.asarray(W_int, dtype=np.float32)
        .astype(F8NP)
        .T.reshape(DT, P, NOC, OC)
        .transpose(2, 1, 0, 3)
    )
    A_aug = np.concatenate(
        [
            np.asarray(lora_A, dtype=np.float32),
            np.ones((1, D), np.float32),
        ],
        axis=0,
    )  # [RT, D]

    def pack_a(v):  # [R, D] -> [P, DT, R]
        R = v.shape[0]
        return np.ascontiguousarray(v.T.reshape(DT, P, R).transpose(1, 0, 2).astype(F8NP))

    A_hi = A_aug.astype(F8NP).astype(np.float32)
    A_lo16 = ((A_aug - A_hi) * 16.0).astype(F8NP).astype(np.float32)
    # pu row layout: [u_top(RT); u_top dup(RT); u_bot(RT); pad to RB]
    a2 = pack_a(
        np.concatenate(
            [A_hi, A_hi, A_lo16, np.zeros((RB - R3, D), np.float32)], axis=0
        )
    )
    a16 = pack_a(
        np.concatenate(
            [A_hi / 16.0, A_hi / 16.0, np.zeros((RC - R2, D), np.float32)],
            axis=0,
        )
    )
    # dual-fp8 B-side [RB, 2, O]: (p, j) slots pair with u8 as
    #   rows 0:RT   j=0: uh*Bh     j=1: ul*Bh
    #   rows RT:R2  j=0: uh2*Bl    j=1: 0
    #   rows R2:R3  j=0: ub*(Bh/16) j=1: 0      (u_bot carries a 16x)
    Bp = np.concatenate(
        [
            np.asarray(lora_B, dtype=np.float32).T * (SCALING / s),
            np.full((1, O), -zp, np.float32),
        ],
        axis=0,
    )  # [RT, O]
    Bh8 = Bp.astype(F8NP)
    Bhf = Bh8.astype(np.float32)
    Bl8 = (Bp - Bhf).astype(F8NP)
    bts8 = np.zeros((RB, 2, O), F8NP)
    bts8[0:RT, 0] = Bh8
    bts8[0:RT, 1] = Bh8
    bts8[RT:R2, 0] = Bl8
    bts8[R2:R3, 0] = (Bhf / 16.0).astype(F8NP)
    bts8 = np.ascontiguousarray(bts8)
    in_maps = []
    for c in range(N_CORES):
        xs = xf[c * T : (c + 1) * T]  # [T, D] f32
        xhi8 = xs.astype(F8NP)
        xlo8 = ((xs - xhi8.astype(np.float32)) * 16.0).astype(F8NP)
        in_maps.append(
            {
                "xhi": pack_x(xhi8),
                "xlo": pack_x_hb(xlo8),
                "w8c": w8c,
                "a2": a2,
                "a16": a16,
                "bts8": bts8,
            }
        )
    return in_maps, T, D, O


def _install_ntff_shim():
    """Provide antenv.axon_hooks (absent in this image) so that
    run_bass_kernel_spmd(trace=True) can capture NTFF profiles via the
    axon .so — mirrors trn_agent_boot.trn_boot's degraded-silently path.
    Only used for our own measurement runs (_trace=True)."""
    import sys as _sys
    import types as _types

    if "antenv.axon_hooks" in _sys.modules:
        return
    try:
        from trn_agent_boot.trn_boot import _ntff_profile_via_ctypes
    except ImportError:
        _sys.path.insert(0, "/root/.axon_site")
        from trn_agent_boot.trn_boot import _ntff_profile_via_ctypes

    hook = _ntff_profile_via_ctypes("/opt/axon/libaxon_pjrt.so")
    mod = _types.ModuleType("antenv.axon_hooks")
    mod._hook = hook
    mod.get_axon_ntff_profile_hook = lambda: mod._hook
    mod.set_axon_ntff_profile_hook = lambda h: setattr(mod, "_hook", h)
    _sys.modules["antenv.axon_hooks"] = mod
    import antenv as _antenv

    _antenv.axon_hooks = mod


def kernel(x, W_int, lora_A, lora_B, scale, zero_point, _trace=False, _tmpdir=None):
    if _trace:
        _install_ntff_shim()
    x = np.asarray(x)
    BS, S, D = x.shape
    s = float(np.asarray(scale))
    zp = float(np.asarray(zero_point))
    in_maps, T, D, O = _pack_inputs(x, W_int, lora_A, lora_B, s, zp)

    nc = bacc.Bacc(
        "TRN2",
        target_bir_lowering=False,
        debug=False,
        num_devices=N_CORES,
    )
    build_program(nc, T, D, O, scale=s)
    nc.compile()

    res = run_bass_kernel_spmd(
        nc,
        in_maps,
        core_ids=list(range(N_CORES)),
        trace=_trace,
        tmpdir=_tmpdir,
        trace_cores=list(range(N_CORES)) if _trace else None,
    )
    y = (
        np.concatenate([np.asarray(r["y"]) for r in res.results], axis=0)
        .astype(np.float32)
        .reshape(BS, S, O)
    )
    if _trace:
        kernel.last_results = res
    return y


if __name__ == "__main__":
    # smoke: build-only for full shapes
    nc = bacc.Bacc("TRN2", target_bir_lowering=False, debug=False, num_devices=8)
    build_program(nc, 1024, 4096, 4096, scale=0.01)
    nc.compile()
    print("build ok; instructions:", sum(len(b.instructions) for b in nc.main_func.blocks))


# revision 25
# speedup vs baseline: 1.0157x; 1.0033x over previous
# LoftQ fused kernel for Trainium2 (Bass/Tile), 8-core data-parallel, fp8.
#
# reference:
#   W_q = (W_int - zero_point) * scale                  [out=4096, in=4096]
#   W   = W_q + (lora_B @ lora_A) * RANK**-0.5
#   y   = einsum('bsd,od->bso', x, W)                   x: [4, 2048, 4096]
#
# Strategy:
#   - Data-parallel: 8192 tokens sharded 1024/core; W replicated.
#   - Decompose y = s*(x @ W_int.T) - s*zp*rowsum(x) + (x @ A.T) @ (sc*B.T)
#     W_int values 0..15 are EXACT in fp8e4m3, so the main GEMM runs as
#     fp8 x fp8 with MatmulPerfMode.DoubleRow (K=256 per instruction,
#     ~216ns issue rate per [256x128x512] matmul; ~245us of PE work/core).
#   - x is split hi/lo: xhi = f8(x), xlo = f8(16*(x - xhi)). The main GEMM
#     uses xhi only (its error lands on the small quantized term; the
#     LoRA term dominates output variance). The LoRA path u = x @ A_aug.T
#     runs as fp8 DoubleRow chains: chain12 = xhi @ [Ahi ; 16*(A-Ahi)]
#     plus chain3 = xlo @ (Ahi/16), giving ~bf16 accuracy.
#   - zero point folded in by augmenting A with a ones row; bts row 16 =
#     -zp. A K=64 bf16 tail matmul per oc-tile adds u @ (sc*B.T/s | -zp)
#     into the main PSUM group; eviction scales by s.
#
# DMA system model (from trace archaeology): two HWDGE rings (sync=SP,
# scalar=ACT) issue in program order, ~0.6us engine time per dma_start,
# sharing 8 global completion-semaphore lanes with ONE outstanding DMA
# per lane (round-robin in combined issue order) and ~1.5us fixed
# latency per piece; aggregate effective bandwidth during bursts is
# ~270-330 GB/s. Choreography is therefore:
#   - Ring split by role: sync ring = x-side + y writebacks, scalar ring
#     = W only, so W pieces are never head-of-line blocked behind x
#     queue-drain waits (that caused an 8.3us PE stall at a W-group
#     boundary when everything shared one ring).
#   - oc-chunk-major main loop (one 512-wide PSUM bank per tile, 6 in
#     flight): the startup only needs ONE 2MB W chunk, and each W chunk
#     thereafter has a ~30us landing window (67 GB/s steady demand).
#   - Startup interleave: per dt2 block the PE runs chain12 (both token
#     halves), chain3 for token-half 0 (lagged 4 blocks so xlo pieces
#     have landing slack), and the first 6 token-tiles' DR steps for oc
#     chunk 0 — consuming x/W pieces as they land (~270 GB/s demand vs
#     the ~1.9us/block PE pace) instead of idling until xhi is resident.
#   - xlo is packed token-half-major; chain3 for half 1 closes later,
#     behind the half-0 tails.
#   - Pieces are deadline-ordered and sized ~256-512KB-1MB (small enough
#     to pace consumption, big enough to amortize the per-piece latency;
#     the first xhi block is a single piece so no lane straggler gates
#     the first matmul).
#   - W chunks 1+ are gated behind xlo landings (a dummy scalar read of
#     the last xlo piece) so startup SDMA bandwidth goes to x; chunks 3+
#     auto-gate on W-pool buffer reuse (3 resident 2MB buffers).
#   - All PSUM evictions run on the vector engine (scalar is the W DMA
#     issuer). The last tile's writeback is serialized and split across
#     both HWDGE rings to shorten the drain.
#
# Host-side work is limited to sharding/layout packing (transpose + dtype
# packing); all FLOPs (both matmuls, dequant-by-linearity) run on device.

import numpy as np
import ml_dtypes

import concourse.bass as bass
import concourse.mybir as mybir
import concourse.tile as tile
from concourse import bacc
from concourse.bass import ts
from concourse.bass_utils import run_bass_kernel_spmd

P = 128
N_CORES = 8
RANK = 16
# u-phase PSUM row layout: [u_top(17); u_top duplicate(17); u_bot(17); pad]
# - u_top = x @ A_aug.T (A augmented with a ones row for the zp rowsum)
# - the duplicate rows are free (PE cost is moving-row-bound) and make
#   every fp8 eviction slice partition-aligned for the dual-fp8 tail
# - u_bot = xhi @ (16*(A - A_hi)).T, the A-quantization correction
RT = RANK + 1
R2 = 2 * RT
R3 = 3 * RT
RB = 64   # u-phase PSUM rows
RC = 48   # chain3 stationary cols: R2 padded — dual-fp8 ldweights needs
          # the k-pair stride (the lhsT column count) % 16 == 0
KB = 128  # tail stationary partitions: full 128 measures ~100ns faster
          # per tail MM than a 64-partition stationary (same ap cost)
SCALING = RANK ** (-0.5)
BF16 = mybir.dt.bfloat16
F32 = mybir.dt.float32
F8 = mybir.dt.float8e4
OC = 512      # output-feature chunk (one PSUM bank wide)

DR = mybir.MatmulPerfMode.DoubleRow
COPY = mybir.ActivationFunctionType.Copy
MULT = mybir.AluOpType.mult


def build_program(nc, T, D, O, scale):
    """Emit the per-core program.

    T: tokens per core, D: in_features, O: out_features.
    Inputs (per core):
      xhi  f8e4 [P, D/P, T]        f8(x) shard, transposed+partition-packed
      xlo  f8e4 [P, NHB, D/P, UW]  f8(16*(x - xhi)), token-half-major
      w8c  f8e4 [NOC, P, D/P, OC]  W_int^T chunk-packed (replicated)
      a2   f8e4 [P, D/P, RB]  [Ahi(17) ; Ahi(17) ; 16*(A_aug-Ahi)(17) ; 0]
      a16  f8e4 [P, D/P, RC]  [Ahi/16(17) ; Ahi/16(17) ; 0-pad] (replicated)
      bts8 f8e4 [KB, 2, O]    dual-fp8 B-side for the DR tail (replicated)
    Output: y bf16 [T, O]  (host casts to f32); y = scale * psum
    """
    DT, TT, NOC = D // P, T // P, O // OC
    D2 = DT // 2
    UW = min(512, T)     # u-phase moving width (one token half)
    NHB = T // UW
    TPH = UW // P        # token tiles per half
    assert DT % 2 == 0

    xhi = nc.dram_tensor("xhi", [P, DT, T], F8, kind="ExternalInput")
    xlo = nc.dram_tensor("xlo", [P, NHB, DT, UW], F8, kind="ExternalInput")
    w8 = nc.dram_tensor("w8c", [NOC, P, DT, OC], F8, kind="ExternalInput")
    a2 = nc.dram_tensor("a2", [P, DT, RB], F8, kind="ExternalInput")
    a16 = nc.dram_tensor("a16", [P, DT, RC], F8, kind="ExternalInput")
    bts8 = nc.dram_tensor("bts8", [RB, 2, O], F8, kind="ExternalInput")
    y = nc.dram_tensor("y", [T, O], BF16, kind="ExternalOutput")
    y_ap = y.ap().rearrange("(tt p) o -> p tt o", p=P)

    iters = [(oc, tt) for oc in range(NOC) for tt in range(TT)]
    PRE = max(1, min(6, len(iters) - 1))
    CH3_LAG = min(6, max(1, D2 - 1))

    with tile.TileContext(nc) as tc:
        with (
            tc.tile_pool(name="const", bufs=1) as cpool,
            tc.tile_pool(name="wc", bufs=min(3, NOC)) as wcpool,
            tc.tile_pool(name="outpool", bufs=4) as outpool,
            tc.tile_pool(name="psum", bufs=6, space="PSUM") as psum,
            tc.tile_pool(name="psum_u", bufs=NHB, space="PSUM") as psum_u,
        ):
            a2_sb = cpool.tile([P, DT, RB], F8)
            xhi_sb = cpool.tile([P, DT, T], F8)
            xlo_sb = cpool.tile([P, NHB, DT, UW], F8)
            a16_sb = cpool.tile([P, DT, RC], F8)
            bts8_sb = cpool.tile([KB, 2, O], F8)
            u8_sb = cpool.tile([KB, 2, T], F8)
            uhf_sb = cpool.tile([RT, UW], F32)
            uc_sb = cpool.tile([R3, UW], F32)
            gate_sb = cpool.tile([1, 16], BF16)
            # zero the u8 pad slots once (a stale NaN times a zero bts8
            # row would poison the tail accumulation); bts8's upper rows
            # are zero too and never DMA'd (saves 512KB in the startup
            # bandwidth crunch)
            nc.gpsimd.memset(u8_sb[:], 0.0)
            nc.gpsimd.memset(bts8_sb[RB:KB], 0.0)
            wc_sb = [
                wcpool.tile([P, DT, OC], F8, tag="wc", name=f"wc_{oc}")
                for oc in range(NOC)
            ]

            # ---- sync (SP) ring: x-side inputs, deadline-ordered ----
            # xhi's first block is the true gate for the first matmul;
            # the tiny a2 piece lands fast from any slot
            nc.sync.dma_start(xhi_sb[:, 0:2], xhi.ap()[:, 0:2])
            nc.sync.dma_start(a2_sb[:, 0:2], a2.ap()[:, 0:2])
            nc.sync.dma_start(a16_sb[:], a16.ap())
            if DT > 2:
                nc.sync.dma_start(a2_sb[:, 2:DT], a2.ap()[:, 2:DT])
                nc.sync.dma_start(xhi_sb[:, 2:4], xhi.ap()[:, 2:4])
            # remaining xhi in 4-dt pieces, interleaved with xlo-half-0
            # pieces so both streams meet the phase-S consumption pace
            xhp = [(d, min(d + 4, DT)) for d in range(4, DT, 4)]
            NXP = 4 if DT % 8 == 0 else 1
            XD = DT // NXP

            def xlo_pieces(hb):
                return [(hb, r * XD, (r + 1) * XD) for r in range(NXP)]

            lq = list(xlo_pieces(0))
            # deadline order: chain3 lags 8 blocks, so xlo-h0 pieces slot
            # in after every third xhi piece (xhi+W gate 6 of 8 MMs/step)
            for i, (d0, d1) in enumerate(xhp):
                nc.sync.dma_start(xhi_sb[:, d0:d1], xhi.ap()[:, d0:d1])
                if i % 2 == 1 and lq:
                    hb, l0, l1 = lq.pop(0)
                    nc.sync.dma_start(
                        xlo_sb[:, hb, l0:l1], xlo.ap()[:, hb, l0:l1]
                    )
            for hb, l0, l1 in lq:
                nc.sync.dma_start(xlo_sb[:, hb, l0:l1], xlo.ap()[:, hb, l0:l1])
            nc.sync.dma_start(bts8_sb[0:RB], bts8.ap())
            for hb in range(1, NHB):
                for _, l0, l1 in xlo_pieces(hb):
                    nc.sync.dma_start(
                        xlo_sb[:, hb, l0:l1], xlo.ap()[:, hb, l0:l1]
                    )

            # ---- scalar (ACT) ring: W only ----
            pre_ocs = []
            for oc, _ in iters[:PRE]:
                if oc not in pre_ocs:
                    pre_ocs.append(oc)
            rest_ocs = []
            for oc, _ in iters[PRE:]:
                if oc not in pre_ocs and oc not in rest_ocs:
                    rest_ocs.append(oc)
            NWP = 4 if DT % 8 == 0 else 1
            WD = DT // NWP
            # hold W off the initial HBM burst: the first tile DR needs
            # W only at ~+14us, while xhi's first block gates everything
            nc.scalar.activation(gate_sb[:], a2_sb[0:1, 1, 0:16], COPY)
            for oc in pre_ocs:
                for r in range(NWP):
                    nc.scalar.dma_start(
                        wc_sb[oc][:, r * WD : (r + 1) * WD],
                        w8.ap()[oc, :, r * WD : (r + 1) * WD],
                    )
            for k, oc in enumerate(rest_ocs):
                if k == 0:
                    # gate: consume the last piece of xlo-half-0
                    nc.scalar.activation(
                        gate_sb[:], xlo_sb[0:1, 0, DT - 1, 0:16], COPY
                    )
                if k == 1 and NHB > 1:
                    nc.scalar.activation(
                        gate_sb[:], xlo_sb[0:1, NHB - 1, DT - 1, 0:16], COPY
                    )
                for h in range(2):
                    d0, d1 = h * DT // 2, (h + 1) * DT // 2
                    nc.scalar.dma_start(
                        wc_sb[oc][:, d0:d1], w8.ap()[oc, :, d0:d1]
                    )

            # ---- PE program ----
            pu = [
                psum_u.tile([RB, UW], F32, tag="pu", name=f"pu_{hb}")
                for hb in range(NHB)
            ]
            ps = {}

            def u12_mm(dt2, hb, stop=False):
                nc.tensor.matmul(
                    pu[hb][:RB],
                    lhsT=a2_sb[:, 2 * dt2 : 2 * dt2 + 2],
                    rhs=xhi_sb[:, 2 * dt2 : 2 * dt2 + 2, ts(hb, UW)],
                    start=(dt2 == 0),
                    stop=stop,
                    perf_mode=DR,
                )

            def u3_mm(hb, dt2):
                nc.tensor.matmul(
                    pu[hb][:RC],
                    lhsT=a16_sb[:, 2 * dt2 : 2 * dt2 + 2],
                    rhs=xlo_sb[:, hb, 2 * dt2 : 2 * dt2 + 2],
                    start=False,
                    stop=False,
                    perf_mode=DR,
                )

            def prep_u8(hb):
                # dual-fp8 u for the DR tail: one partition-aligned copy
                # covers u_top-hi, its duplicate, and u_bot; the residual
                # ul = u_top - f8(u_top) fills the pair slot of rows 0:RT.
                # TRN fp8e4 overflows to inf above +-240 (not the OCP 448),
                # so clamp before the downcast; the ul residual is computed
                # against the clamped+rounded value, so it absorbs the
                # clamp error exactly on the uh*Bh path.
                hs = ts(hb, UW)
                nc.vector.tensor_scalar_min(uc_sb[:], pu[hb][0:R3], 224.0)
                nc.vector.tensor_scalar_max(uc_sb[:], uc_sb[:], -224.0)
                nc.vector.tensor_scalar(
                    u8_sb[0:R3, 0, hs], uc_sb[:], 1.0, None, MULT
                )
                nc.vector.tensor_scalar(
                    uhf_sb[:], u8_sb[0:RT, 0, hs], 1.0, None, MULT
                )
                nc.vector.tensor_tensor(
                    u8_sb[0:RT, 1, hs], pu[hb][0:RT], uhf_sb[:],
                    mybir.AluOpType.subtract,
                )

            def emit_dr_step(oc, tt, dt2):
                if dt2 == 0:
                    ps[oc, tt] = psum.tile(
                        [P, OC], F32, tag="ps", name=f"ps_{oc}_{tt}"
                    )
                nc.tensor.matmul(
                    ps[oc, tt][:],
                    lhsT=xhi_sb[:, 2 * dt2 : 2 * dt2 + 2, ts(tt, P)],
                    rhs=wc_sb[oc][:, 2 * dt2 : 2 * dt2 + 2],
                    start=(dt2 == 0),
                    stop=False,
                    perf_mode=DR,
                )

            def emit_drs(oc, tt):
                for dt2 in range(D2):
                    emit_dr_step(oc, tt, dt2)

            def emit_tail_evict(oc, tt, nsp, alt_ring=False):
                nc.tensor.matmul(
                    ps[oc, tt][:],
                    lhsT=u8_sb[:, :, ts(tt, P)],
                    rhs=bts8_sb[:, :, ts(oc, OC)],
                    start=False,
                    stop=True,
                    perf_mode=DR,
                )
                ob = outpool.tile([P, OC], BF16, tag="ob", name=f"ob_{oc}_{tt}")
                nc.vector.tensor_scalar(ob[:], ps[oc, tt][:], scale, None, MULT)
                for q in range(nsp):
                    pr = ts(q, P // nsp)
                    eng = nc.scalar if (alt_ring and (oc + tt) % 2) else nc.sync
                    eng.dma_start(y_ap[pr, tt, ts(oc, OC)], ob[pr])

            # Startup interleave: per dt2 block the u chains and the first
            # PRE tiles' DR steps consume x/W pieces as each block lands.
            for dt2 in range(D2):
                if dt2 < D2 - 1:
                    for hb in range(NHB):
                        u12_mm(dt2, hb)
                if dt2 >= CH3_LAG:
                    u3_mm(0, dt2 - CH3_LAG)
                for oc, tt in iters[:PRE]:
                    emit_dr_step(oc, tt, dt2)
            for dt2 in range(D2 - CH3_LAG, D2):
                u3_mm(0, dt2)
            u12_mm(D2 - 1, 0, stop=True)
            prep_u8(0)
            # token-half-0 tails first; close half 1 behind them
            for oc, tt in iters[:PRE]:
                if tt < TPH:
                    emit_tail_evict(oc, tt, 1)
            for hb in range(1, NHB):
                for dt2 in range(D2):
                    u3_mm(hb, dt2)
                u12_mm(D2 - 1, hb, stop=True)
                prep_u8(hb)
            for oc, tt in iters[:PRE]:
                if tt >= TPH:
                    emit_tail_evict(oc, tt, 1)
            # steady state: batch 6 tiles' DR chains then their 6 tails —
            # fewer chunk boundaries (~400ns ldweights refill each); the
            # first DR of chunk i waits only on chunk i-1's first
            # eviction, which completes while the remaining tails issue
            GRP = 6
            rest = iters[PRE:-1]
            for c0 in range(0, len(rest), GRP):
                chunk = rest[c0 : c0 + GRP]
                for oc, tt in chunk:
                    emit_drs(oc, tt)
                for oc, tt in chunk:
                    emit_tail_evict(oc, tt, 1)
            # final tile: split into two 256-wide accumulation groups
            # (disjoint PSUM column regions accumulate independently) so
            # the left half's eviction+writeback overlaps the right
            # half's DR chain — only ~2.5us of writeback chain remains
            # after the very last matmul.
            if PRE < len(iters):
                ocL, ttL = iters[-1]
                HO = OC // 2
                ps[ocL, ttL] = psum.tile(
                    [P, OC], F32, tag="ps", name=f"ps_{ocL}_{ttL}"
                )
                ob = outpool.tile([P, OC], BF16, tag="ob", name="ob_last")
                for h in range(2):
                    co = h * HO
                    for dt2 in range(D2):
                        nc.tensor.matmul(
                            ps[ocL, ttL][:, co : co + HO],
                            lhsT=xhi_sb[:, 2 * dt2 : 2 * dt2 + 2, ts(ttL, P)],
                            rhs=wc_sb[ocL][
                                :, 2 * dt2 : 2 * dt2 + 2, co : co + HO
                            ],
                            start=(dt2 == 0),
                            stop=False,
                            perf_mode=DR,
                        )
                    nc.tensor.matmul(
                        ps[ocL, ttL][:, co : co + HO],
                        lhsT=u8_sb[:, :, ts(ttL, P)],
                        rhs=bts8_sb[:, :, ocL * OC + co : ocL * OC + co + HO],
                        start=False,
                        stop=True,
                        perf_mode=DR,
                    )
                    nc.vector.tensor_scalar(
                        ob[:, co : co + HO],
                        ps[ocL, ttL][:, co : co + HO],
                        scale,
                        None,
                        MULT,
                    )
                    for q in range(2):
                        pr = ts(q, P // 2)
                        eng = nc.sync if q == 0 else nc.scalar
                        eng.dma_start(
                            y_ap[pr, ttL, ocL * OC + co : ocL * OC + co + HO],
                            ob[pr, co : co + HO],
                        )
    return nc


def _pack_inputs(x, W_int, lora_A, lora_B, scale, zero_point):
    """Host-side shard + layout packing. Returns per-core input maps."""
    F8NP = ml_dtypes.float8_e4m3
    BFNP = ml_dtypes.bfloat16
    BS, S, D = x.shape
    O = W_int.shape[0]
    Tfull = BS * S
    T = Tfull // N_CORES
    DT = D // P
    NOC = O // OC
    UW = min(512, T)
    NHB = T // UW
    s = float(scale)
    zp = float(zero_point)

    def pack_x(v):  # [T, D] -> [P, DT, T]
        return np.ascontiguousarray(v.T.reshape(DT, P, T).transpose(1, 0, 2))

    def pack_x_hb(v):  # [T, D] -> [P, NHB, DT, UW] token-half-major
        return np.ascontiguousarray(
            v.reshape(NHB, UW, DT, P).transpose(3, 0, 2, 1)
        )

    xf = np.asarray(x, dtype=np.float32).reshape(Tfull, D)
    # [oc, p, dt, j] <- W_int^T[d=dt*P+p, o=oc*OC+j], exact in fp8e4m3
    w8c = np.ascontiguousarray(
        np.asarray(W_int, dtype=np.float32)
        .astype(F8NP)
        .T.reshape(DT, P, NOC, OC)
        .transpose(2, 1, 0, 3)
    )
    A_aug = np.concatenate(
        [
            np.asarray(lora_A, dtype=np.float32),
            np.ones((1, D), np.float32),
        ],
        axis=0,
    )  # [RT, D]

    def pack_a(v):  # [R, D] -> [P, DT, R]
        R = v.shape[0]
        return np.ascontiguousarray(v.T.reshape(DT, P, R).transpose(1, 0, 2).astype(F8NP))

    A_hi = A_aug.astype(F8NP).astype(np.float32)
    A_lo16 = ((A_aug - A_hi) * 16.0).astype(F8NP).astype(np.float32)
    # pu row layout: [u_top(RT); u_top dup(RT); u_bot(RT); pad to RB]
    a2 = pack_a(
        np.concatenate(
            [A_hi, A_hi, A_lo16, np.zeros((RB - R3, D), np.float32)], axis=0
        )
    )
    a16 = pack_a(
        np.concatenate(
            [A_hi / 16.0, A_hi / 16.0, np.zeros((RC - R2, D), np.float32)],
            axis=0,
        )
    )
    # dual-fp8 B-side [RB, 2, O]: (p, j) slots pair with u8 as
    #   rows 0:RT   j=0: uh*Bh     j=1: ul*Bh
    #   rows RT:R2  j=0: uh2*Bl    j=1: 0
    #   rows R2:R3  j=0: ub*(Bh/16) j=1: 0      (u_bot carries a 16x)
    Bp = np.concatenate(
        [
            np.asarray(lora_B, dtype=np.float32).T * (SCALING / s),
            np.full((1, O), -zp, np.float32),
        ],
        axis=0,
    )  # [RT, O]
    Bh8 = Bp.astype(F8NP)
    Bhf = Bh8.astype(np.float32)
    Bl8 = (Bp - Bhf).astype(F8NP)
    bts8 = np.zeros((RB, 2, O), F8NP)
    bts8[0:RT, 0] = Bh8
    bts8[0:RT, 1] = Bh8
    bts8[RT:R2, 0] = Bl8
    bts8[R2:R3, 0] = (Bhf / 16.0).astype(F8NP)
    bts8 = np.ascontiguousarray(bts8)
    in_maps = []
    for c in range(N_CORES):
        xs = xf[c * T : (c + 1) * T]  # [T, D] f32
        xhi8 = xs.astype(F8NP)
        xlo8 = ((xs - xhi8.astype(np.float32)) * 16.0).astype(F8NP)
        in_maps.append(
            {
                "xhi": pack_x(xhi8),
                "xlo": pack_x_hb(xlo8),
                "w8c": w8c,
                "a2": a2,
                "a16": a16,
                "bts8": bts8,
            }
        )
    return in_maps, T, D, O


def _install_ntff_shim():
    """Provide antenv.axon_hooks (absent in this image) so that
    run_bass_kernel_spmd(trace=True) can capture NTFF profiles via the
    axon .so — mirrors trn_agent_boot.trn_boot's degraded-silently path.
    Only used for our own measurement runs (_trace=True)."""
    import sys as _sys
    import types as _types

    if "antenv.axon_hooks" in _sys.modules:
        return
    try:
        from trn_agent_boot.trn_boot import _ntff_profile_via_ctypes
    except ImportError:
        _sys.path.insert(0, "/root/.axon_site")
        from trn_agent_boot.trn_boot import _ntff_profile_via_ctypes

    hook = _ntff_profile_via_ctypes("/opt/axon/libaxon_pjrt.so")
    mod = _types.ModuleType("antenv.axon_hooks")
    mod._hook = hook
    mod.get_axon_ntff_profile_hook = lambda: mod._hook
    mod.set_axon_ntff_profile_hook = lambda h: setattr(mod, "_hook", h)
    _sys.modules["antenv.axon_hooks"] = mod
    import antenv as _antenv

    _antenv.axon_hooks = mod


def kernel(x, W_int, lora_A, lora_B, scale, zero_point, _trace=False, _tmpdir=None):
    if _trace:
        _install_ntff_shim()
    x = np.asarray(x)
    BS, S, D = x.shape
    s = float(np.asarray(scale))
    zp = float(np.asarray(zero_point))
    in_maps, T, D, O = _pack_inputs(x, W_int, lora_A, lora_B, s, zp)

    nc = bacc.Bacc(
        "TRN2",
        target_bir_lowering=False,
        debug=False,
        num_devices=N_CORES,
    )
    build_program(nc, T, D, O, scale=s)
    nc.compile()

    res = run_bass_kernel_spmd(
        nc,
        in_maps,
        core_ids=list(range(N_CORES)),
        trace=_trace,
        tmpdir=_tmpdir,
        trace_cores=list(range(N_CORES)) if _trace else None,
    )
    y = (
        np.concatenate([np.asarray(r["y"]) for r in res.results], axis=0)
        .astype(np.float32)
        .reshape(BS, S, O)
    )
    if _trace:
        kernel.last_results = res
    return y


if __name__ == "__main__":
    # smoke: build-only for full shapes
    nc = bacc.Bacc("TRN2", target_bir_lowering=False, debug=False, num_devices=8)
    build_program(nc, 1024, 4096, 4096, scale=0.01)
    nc.compile()
    print("build ok; instructions:", sum(len(b.instructions) for b in nc.main_func.blocks))
